# revision 56
# baseline (speedup 1.0000x reference)
"""S4D "CopyingModel" Trainium2 Bass kernel (V2).

Math: logits = (s4d_scan(emb[x]) + emb[x]*D) @ W_out + b_out, with a
per-channel diagonal SSM (d_model=1024 channels, d_state=64).

Strategy (8 NeuronCores, channel-sharded: 128 channels per core, every core
sees all 8 batches x 4096 tokens):
  - host precomputes (f64) discretized per-channel operators, fp16 on device:
      * T[d]  [L,L]  lower-tri Toeplitz of the conv kernel K (D skip folded)
      * E[d]  [L,N]  chunk-end state accumulator dA^(L-1-j)*dB
      * Cb[d] [N,L]  state->output C*dA^(i+1)
      * Pm    [p,q,33] chunk decay dA^L with a 0 reset column (segmented scan)
  - token/tile index t = b*32 + c (batch-major, chunk inner), chunk L=128
  - embedding gather = one-hot matmul; u and y share one SBUF buffer
    (y overwrites u channel pairs as phase D drains them)
  - chunk-carry scan = 8 DVE tensor_tensor_scan ops (segmented via Pm zeros)
  - output projection from PE-transposed y tiles; partials written as bf16;
    host sums cores + bias.

vs the original baseline: PSUM drains batched to full banks (4-8x fewer
DVE/Act copies), the 62-op elementwise chunk scan replaced by segmented
tensor_tensor_scan instructions split per (batch, q-half) and interleaved
with phase B so they hide under B's tail and D's T-matmuls, DMA issue
spread across SP (HWDGE) and Pool (SWDGE) queues with weight prefetch
emitted ahead of the Pool-queue scans, v-stacked output projection (two
64-row logit tiles per 128-partition PSUM bank via tile_position), u/y
sharing one SBUF buffer, and bf16 output partials (half the out DMA).
"""

import os
from contextlib import ExitStack

import numpy as np

BATCH = 8
SEQ = 4096
D_MODEL = 1024
N_STATE = 64
VOCAB = 64
L = 128                   # chunk length
NCH = SEQ // L            # 32 chunks
NCORES = 8
DPC = D_MODEL // NCORES   # 128 channels per core
BC = NCH * BATCH          # 256 (batch, chunk) tiles; t = b*NCH + c

GT = 8    # T channels per DMA group
GE = 16   # E channels per DMA group
GQ = 8    # Cb channel-pairs per DMA group

LAST_RESULTS = None       # BassKernelResults of the most recent run (for test.py)


def _precompute_host(emb, log_neg_A, Bmat, C, Dvec, log_dt, W_out):
    """Float64 host precompute of all device operands."""
    dt = np.exp(log_dt.astype(np.float64))                    # (D,)
    A = -np.exp(log_neg_A.astype(np.float64))                 # (D,N)
    dA = np.exp(dt[:, None] * A)                              # (D,N)
    dB = (dA - 1.0) / A * Bmat.astype(np.float64)             # (D,N)
    w = C.astype(np.float64) * dB                             # (D,N)

    # dApow[d,n,k] = dA^k, k=0..L-1
    dApow = np.ones((D_MODEL, N_STATE, L))
    np.cumprod(np.broadcast_to(dA[:, :, None], (D_MODEL, N_STATE, L - 1)),
               axis=2, out=dApow[:, :, 1:])
    K = np.einsum("dn,dnk->dk", w, dApow)                     # (D,L)
    K[:, 0] += Dvec.astype(np.float64)                        # fold skip

    # Toeplitz lhsT: T[d][j,i] = K[d, i-j] for i>=j
    T = np.zeros((D_MODEL, L, L), np.float32)
    Kf = K.astype(np.float32)
    for k in range(L):
        idx = np.arange(L - k)
        T[:, idx, idx + k] = Kf[:, k][:, None]

    # E lhsT [d, j, n] = dA^(L-1-j) * dB
    E = (dApow[:, :, ::-1] * dB[:, :, None]).transpose(0, 2, 1)  # (D,L,N)
    # Cb lhsT [d, n, i] = C * dA^(i+1)
    dApow1 = dApow * dA[:, :, None]
    Cb = C.astype(np.float64)[:, :, None] * dApow1               # (D,N,L)
    P = dApow1[:, :, L - 1]                                      # dA^L (D,N)
    return (T.astype(np.float16), E.astype(np.float16),
            Cb.astype(np.float16), P)


def _emit_kernel(nc, tile, mybir, make_identity):
    f16 = mybir.dt.float16
    f32 = mybir.dt.float32
    bf16 = mybir.dt.bfloat16

    onehotT = nc.dram_tensor("onehot_t", [VOCAB, BC * L], f16,
                             kind="ExternalInput").ap()
    embs = nc.dram_tensor("emb_s", [VOCAB, DPC], f16, kind="ExternalInput").ap()
    # [group, j, ch_in_group, i]
    t_all = nc.dram_tensor("t_all", [DPC // GT, L, GT, L], f16,
                           kind="ExternalInput").ap()
    e_all = nc.dram_tensor("e_all", [DPC // GE, L, GE, N_STATE], f16,
                           kind="ExternalInput").ap()
    # [group, p=(par,n), q_in_group, i]
    cb_all = nc.dram_tensor("cb_all", [64 // GQ, 128, GQ, L], f16,
                            kind="ExternalInput").ap()
    # Pm[p, q, 0]=0 (segmented-scan reset), Pm[p, q, c>=1] = dA^L
    pm = nc.dram_tensor("pm", [128, 64, NCH + 1], f16,
                        kind="ExternalInput").ap()
    w2 = nc.dram_tensor("w2", [DPC, VOCAB], f16, kind="ExternalInput").ap()
    out_t = nc.dram_tensor("out_t", [VOCAB, BC * L], bf16,
                           kind="ExternalOutput").ap()

    with tile.TileContext(nc) as tc, ExitStack() as ctx:
        persist = ctx.enter_context(tc.tile_pool(name="persist", bufs=1))
        # u and y share this buffer: u[j, b, c, d]; phase D overwrites
        # channel-pair slices with y[i, b, c, dl] after their last u read.
        uy = persist.tile([128, BATCH, NCH, DPC], f16, name="uy")
        # x_src: S[c-2] landing zone (cols 0,1 zero); x_sc: h_start after scan
        x_src = persist.tile([128, BATCH, 64, NCH + 1], f16, name="x_src")
        x_sc = persist.tile([128, BATCH, 64, NCH + 1], f16, name="x_sc")
        pm_sb = persist.tile([128, 64, NCH + 1], f16, name="pm_sb")
        emb_sb = persist.tile([VOCAB, DPC], f16, name="emb_sb")
        w2_sb = persist.tile([DPC, VOCAB], f16, name="w2_sb")
        ident = persist.tile([128, 128], f16, name="ident")

        make_identity(nc, ident)

        def cp(k, out, in_):
            # GPSIMD cannot read PSUM; alternate the two engines that can.
            if k % 2 == 0:
                nc.vector.tensor_copy(out, in_)
            else:
                nc.scalar.copy(out, in_)

        ncp = 0

        # ---- Phase A: embedding (one-hot @ emb slice) -> uy ----
        # Pool-queue order matters: first one-hot tile + emb before anything
        # else so the PE can start immediately; w2/pm/memset follow.
        with tc.tile_pool(name="ohp", bufs=3) as ohp, \
             tc.tile_pool(name="ps_a", bufs=3, space="PSUM") as ps_a:
            # first two groups are small so the PE starts sooner
            groups = [(0, 8), (8, 24)] + [(32 * g, 32) for g in range(1, 8)]
            for g, (t0, gl) in enumerate(groups):
                oh = ohp.tile([VOCAB, gl, L], f16)
                # alternate issue queues so neither sequencer gates phase A
                (nc.sync if g % 2 == 0 else nc.gpsimd).dma_start(
                    out=oh, in_=onehotT[:, t0 * L:(t0 + gl) * L])
                if g == 0:
                    nc.gpsimd.dma_start(out=emb_sb, in_=embs)
                elif g == 1:
                    nc.gpsimd.dma_start(out=pm_sb, in_=pm)
                    nc.gpsimd.memset(x_src[:, :, :, 0:2], 0.0)
                elif g == 2:
                    nc.gpsimd.dma_start(out=w2_sb, in_=w2)
                for i in range(gl):
                    t = t0 + i
                    b, c = t // NCH, t % NCH
                    if t % 8 == 0:              # 2-bank PSUM tile, 8 tiles
                        ups = ps_a.tile([128, 8, DPC], f32)
                    nc.tensor.matmul(ups[:, t % 8, :], lhsT=oh[:, i, :],
                                     rhs=emb_sb, start=True, stop=True)
                    if t % 8 == 7:
                        cp(ncp,
                           uy[:, b, c - 7:c + 1, :], ups)
                        ncp += 1

        # ---- Phase B + C: chunk-end states, with the segmented scan
        # interleaved per q-half so it hides under B's second half and
        # phase D's T-matmuls. ----
        twp = ctx.enter_context(tc.tile_pool(name="twp", bufs=6))
        cbp = ctx.enter_context(tc.tile_pool(name="cbp", bufs=3))
        bctx = ExitStack()
        ewp = bctx.enter_context(tc.tile_pool(name="ewp", bufs=8))
        ps_s = bctx.enter_context(tc.tile_pool(name="ps_s", bufs=4,
                                               space="PSUM"))
        ew_tiles = {}

        def b_dma(g):
            e_w = ewp.tile([L, GE, N_STATE], f16)
            nc.gpsimd.dma_start(out=e_w, in_=e_all[g])
            ew_tiles[g] = e_w

        def b_mms(g, k0, act_bias=False):
            e_w = ew_tiles[g]
            for i in range(GE // 4):            # 2 q (4 channels) per PSUM
                q0 = (g * GE) // 2 + 2 * i
                sp = ps_s.tile([128, 2, BATCH, NCH - 1], f32)
                for k in range(2):
                    for par in range(2):
                        dl = 2 * (q0 + k) + par
                        nc.tensor.matmul(
                            sp[64 * par:64 * (par + 1), k],
                            lhsT=e_w[:, 4 * i + 2 * k + par, :],
                            rhs=uy[:, :, 0:NCH - 1, dl],
                            start=True, stop=True,
                            tile_position=(0, 64 * par) if par else None)
                # while the DVE runs the half-scans, drain mostly via Act
                kk = 1 if act_bias else k0 + i
                cp(kk, x_src[:, :, q0:q0 + 2, 2:NCH + 1],
                   sp.transpose([0, 2, 1, 3]))
            return k0 + GE // 4

        def quarter_scans(k):
            # walrus: TensorScalarPtr is DVE-only. Quarter k covers q
            # 16k..16k+15 and only needs e-groups 2k and 2k+1 done, so each
            # quarter is emitted as early as its data allows and finishes
            # well before phase D's Cb matmuls reach its q range.
            qs = slice(16 * k, 16 * (k + 1))
            for b in range(BATCH):
                nc.vector.tensor_tensor_scan(
                    out=x_sc[:, b, qs].opt(), data0=pm_sb[:, qs].opt(),
                    data1=x_src[:, b, qs].opt(), initial=0.0,
                    op0=mybir.AluOpType.mult, op1=mybir.AluOpType.add)

        for g in range(8):                      # prefetch all E weights
            b_dma(g)
        for g in range(8):
            ncp = b_mms(g, ncp, act_bias=(g >= 2))
            if g % 2 == 1:
                quarter_scans(g // 2)
        # prefetch D weights for q<24 ahead of phase D
        tws, cbs = [], []
        for g in range(6):
            t_w = twp.tile([L, GT, L], f16)
            nc.gpsimd.dma_start(out=t_w, in_=t_all[g])
            tws.append(t_w)
        for g in range(3):
            cb_w = cbp.tile([128, GQ, L], f16)
            nc.gpsimd.dma_start(out=cb_w, in_=cb_all[g])
            cbs.append(cb_w)
        bctx.close()                            # release ewp + ps_s

        # ---- Phase D: y = T^T u (+) Cb^T h_start, overwrites uy ----
        with tc.tile_pool(name="ps_y", bufs=6, space="PSUM") as ps_y:
            for q in range(64):
                if q % (GT // 2) == 0:
                    if q < 24:
                        t_w = tws[q // 4]
                    else:
                        t_w = twp.tile([L, GT, L], f16)
                        nc.gpsimd.dma_start(out=t_w, in_=t_all[(2 * q) // GT])
                if q % GQ == 0:
                    if q < 24:
                        cb_w = cbs[q // 8]
                    else:
                        cb_w = cbp.tile([128, GQ, L], f16)
                        nc.gpsimd.dma_start(out=cb_w, in_=cb_all[q // GQ])
                yp = ps_y.tile([128, 2, BATCH, NCH], f32)
                for par in range(2):
                    # open+close one accumulation group per bank half —
                    # two simultaneously-open groups in one PSUM zero
                    # region are illegal.
                    dl = 2 * q + par
                    nc.tensor.matmul(yp[:, par], lhsT=t_w[:, dl % GT, :],
                                     rhs=uy[:, :, :, dl],
                                     start=True, stop=False)
                    nc.tensor.matmul(
                        yp[:, par],
                        lhsT=cb_w[64 * par:64 * (par + 1), q % GQ, :],
                        rhs=x_sc[64 * par:64 * (par + 1), :, q, 1:NCH + 1],
                        start=False, stop=True,
                        tile_position=(64 * par, 0) if par else None)
                # 3:5 DVE:Act split — the DVE still owes the h1 scans here
                cp(0 if q % 2 == 0 else 1, uy[:, :, :, 2 * q:2 * q + 2],
                   yp.transpose([0, 2, 3, 1]))
                ncp += 1

        # ---- Phase E: transpose y + output projection -> out_t ----
        # Even G: PE transpose via PSUM; odd G: XBAR DMA transpose on the
        # otherwise-idle SP sequencer (SBUF->SBUF, no PSUM drain).
        with tc.tile_pool(name="ytp", bufs=3) as ytp, \
             tc.tile_pool(name="lop", bufs=2) as lop, \
             tc.tile_pool(name="ps_t", bufs=3, space="PSUM") as ps_t, \
             tc.tile_pool(name="ps_o", bufs=2, space="PSUM") as ps_o:
            for G in range(32):                 # 8 t-tiles per group
                yt = ytp.tile([128, 8, 128], f16)
                tp = ps_t.tile([128, 8, 128], f16)
                for k in range(8):
                    t = 8 * G + k
                    b, c = t // NCH, t % NCH
                    nc.tensor.transpose(tp[:, k, :], uy[:, b, c, :], ident)
                cp(ncp, yt, tp)
                ncp += 1
                # v-stacked projection: even G -> PSUM rows 0:64, odd G ->
                # rows 64:128 (weights loaded at PE columns 64:128), so one
                # [128, 1024] copy drains two G-groups of logits.
                if G % 4 == 0:
                    lo = lop.tile([128, 2, 8, L], bf16)   # 32 t
                if G % 2 == 0:
                    po = ps_o.tile([128, 2, 4 * L], f32)  # 2 banks
                sub = G % 2
                for h in range(2):
                    nc.tensor.matmul(po[64 * sub:64 * (sub + 1), h],
                                     lhsT=w2_sb,
                                     rhs=yt[:, 4 * h:4 * h + 4, :],
                                     start=True, stop=True,
                                     tile_position=(0, 64 * sub) if sub
                                     else None)
                if G % 2 == 1:
                    cp(ncp, lo[:, (G // 2) % 2], po)
                    ncp += 1
                if G % 4 == 3:
                    m2 = G // 4
                    # out col = v*32768 + 4096*m2 + 2048*mm + 1024*sub + f
                    full = out_t[:, m2 * 32 * L:(m2 + 1) * 32 * L] \
                        .rearrange("v (mm s f) -> v mm s f", mm=2, s=2)
                    for sub in range(2):
                        eng = nc.sync if sub == 0 else nc.gpsimd
                        eng.dma_start(
                            out=full[:, :, sub, :],
                            in_=lo[64 * sub:64 * (sub + 1)].opt())


def _build_nc():
    import concourse.tile as tile
    from concourse import bacc, mybir
    from concourse.masks import make_identity

    nc = bacc.Bacc(trn_type="TRN2", target_bir_lowering=False, debug=False)
    _emit_kernel(nc, tile, mybir, make_identity)
    nc.compile()
    return nc


_NC_CACHE = None


def _make_in_maps(x, emb, log_neg_A, B, C, D, log_dt, W_out, b_out):
    x = np.asarray(x).astype(np.int64)
    emb = np.asarray(emb, np.float32)
    log_neg_A = np.asarray(log_neg_A, np.float32)
    B_in = np.asarray(B, np.float32)
    C = np.asarray(C, np.float32)
    D_in = np.asarray(D, np.float32)
    log_dt = np.asarray(log_dt, np.float32)
    W_out = np.asarray(W_out, np.float32)

    T, E, Cb, P = _precompute_host(emb, log_neg_A, B_in, C, D_in, log_dt, W_out)

    # one-hot, token order col = (b*NCH + c)*L + j
    toks = x.reshape(BATCH, NCH, L).reshape(-1)
    onehotT = (np.arange(VOCAB)[:, None] == toks[None, :]).astype(np.float16)

    in_maps = []
    for core in range(NCORES):
        ds = slice(core * DPC, (core + 1) * DPC)
        # Pm layout [p=(par,n), q, c']: p = 64*par + n, d = 2*q + par
        Pc = P[ds].reshape(64, 2, N_STATE).transpose(1, 2, 0).reshape(128, 64)
        Pm = np.zeros((128, 64, NCH + 1), np.float16)
        Pm[:, :, 1:] = Pc[:, :, None].astype(np.float16)
        # t_all: [DPC,L,L] -> [DPC/GT, L, GT, L]
        Tc = np.ascontiguousarray(
            T[ds].reshape(DPC // GT, GT, L, L).transpose(0, 2, 1, 3))
        # e_all: [DPC,L,N] -> [DPC/GE, L, GE, N]
        Ec = np.ascontiguousarray(
            E[ds].reshape(DPC // GE, GE, L, N_STATE).transpose(0, 2, 1, 3))
        # cb_all: [DPC,N,L] -> pair-pack [64, 128=(par,n), L] -> groups of GQ
        Cbp = Cb[ds].reshape(64, 2 * N_STATE, L)   # [q, (par,n), L]
        Cbc = np.ascontiguousarray(
            Cbp.reshape(64 // GQ, GQ, 128, L).transpose(0, 2, 1, 3))
        in_maps.append({
            "onehot_t": onehotT,
            "emb_s": np.ascontiguousarray(emb[:, ds]).astype(np.float16),
            "t_all": Tc,
            "e_all": Ec,
            "cb_all": Cbc,
            "pm": Pm,
            "w2": np.ascontiguousarray(W_out[ds]).astype(np.float16),
        })
    return in_maps


def _postprocess(results, b_out):
    logitsT = np.zeros((VOCAB, BC * L), np.float64)
    for r in results:
        logitsT += r["out_t"].astype(np.float64)
    # col = (b*NCH + c)*L + j
    out = logitsT.T.reshape(BATCH, SEQ, VOCAB)
    return (out + np.asarray(b_out).astype(np.float64)).astype(np.float32)


def kernel(x, emb, log_neg_A, B, C, D, log_dt, W_out, b_out):
    global LAST_RESULTS, _NC_CACHE
    from concourse.bass_utils import run_bass_kernel_spmd

    in_maps = _make_in_maps(x, emb, log_neg_A, B, C, D, log_dt, W_out, b_out)

    if _NC_CACHE is None:
        _NC_CACHE = _build_nc()
    nc = _NC_CACHE

    trace = bool(int(os.environ.get("BASS_TRACE", "0") or "0"))
    LAST_RESULTS = run_bass_kernel_spmd(
        nc, in_maps, core_ids=list(range(NCORES)), trace=trace)

    return _postprocess(LAST_RESULTS.results, b_out)


# revision 58
# speedup vs baseline: 1.0066x; 1.0066x over previous
"""S4D "CopyingModel" Trainium2 Bass kernel (V2).

Math: logits = (s4d_scan(emb[x]) + emb[x]*D) @ W_out + b_out, with a
per-channel diagonal SSM (d_model=1024 channels, d_state=64).

Strategy (8 NeuronCores, channel-sharded: 128 channels per core, every core
sees all 8 batches x 4096 tokens):
  - host precomputes (f64) discretized per-channel operators, fp16 on device:
      * T[d]  [L,L]  lower-tri Toeplitz of the conv kernel K (D skip folded)
      * E[d]  [L,N]  chunk-end state accumulator dA^(L-1-j)*dB
      * Cb[d] [N,L]  state->output C*dA^(i+1)
      * Pm    [p,q,33] chunk decay dA^L with a 0 reset column (segmented scan)
  - token/tile index t = b*32 + c (batch-major, chunk inner), chunk L=128
  - embedding gather = one-hot matmul; u and y share one SBUF buffer
    (y overwrites u channel pairs as phase D drains them)
  - chunk-carry scan = 8 DVE tensor_tensor_scan ops (segmented via Pm zeros)
  - output projection from PE-transposed y tiles; partials written as bf16;
    host sums cores + bias.

vs the original baseline: PSUM drains batched to full banks (4-8x fewer
DVE/Act copies), the 62-op elementwise chunk scan replaced by segmented
tensor_tensor_scan instructions split per (batch, q-half) and interleaved
with phase B so they hide under B's tail and D's T-matmuls, DMA issue
spread across SP (HWDGE) and Pool (SWDGE) queues with weight prefetch
emitted ahead of the Pool-queue scans, v-stacked output projection (two
64-row logit tiles per 128-partition PSUM bank via tile_position), u/y
sharing one SBUF buffer, and bf16 output partials (half the out DMA).
"""

import os
from contextlib import ExitStack

import numpy as np

BATCH = 8
SEQ = 4096
D_MODEL = 1024
N_STATE = 64
VOCAB = 64
L = 128                   # chunk length
NCH = SEQ // L            # 32 chunks
NCORES = 8
DPC = D_MODEL // NCORES   # 128 channels per core
BC = NCH * BATCH          # 256 (batch, chunk) tiles; t = b*NCH + c

GT = 8    # T channels per DMA group
GE = 16   # E channels per DMA group
GQ = 8    # Cb channel-pairs per DMA group

LAST_RESULTS = None       # BassKernelResults of the most recent run (for test.py)


def _precompute_host(emb, log_neg_A, Bmat, C, Dvec, log_dt, W_out):
    """Float64 host precompute of all device operands."""
    dt = np.exp(log_dt.astype(np.float64))                    # (D,)
    A = -np.exp(log_neg_A.astype(np.float64))                 # (D,N)
    dA = np.exp(dt[:, None] * A)                              # (D,N)
    dB = (dA - 1.0) / A * Bmat.astype(np.float64)             # (D,N)
    w = C.astype(np.float64) * dB                             # (D,N)

    # dApow[d,n,k] = dA^k, k=0..L-1
    dApow = np.ones((D_MODEL, N_STATE, L))
    np.cumprod(np.broadcast_to(dA[:, :, None], (D_MODEL, N_STATE, L - 1)),
               axis=2, out=dApow[:, :, 1:])
    K = np.einsum("dn,dnk->dk", w, dApow)                     # (D,L)
    K[:, 0] += Dvec.astype(np.float64)                        # fold skip

    # Toeplitz lhsT: T[d][j,i] = K[d, i-j] for i>=j
    T = np.zeros((D_MODEL, L, L), np.float32)
    Kf = K.astype(np.float32)
    for k in range(L):
        idx = np.arange(L - k)
        T[:, idx, idx + k] = Kf[:, k][:, None]

    # E lhsT [d, j, n] = dA^(L-1-j) * dB
    E = (dApow[:, :, ::-1] * dB[:, :, None]).transpose(0, 2, 1)  # (D,L,N)
    # Cb lhsT [d, n, i] = C * dA^(i+1)
    dApow1 = dApow * dA[:, :, None]
    Cb = C.astype(np.float64)[:, :, None] * dApow1               # (D,N,L)
    P = dApow1[:, :, L - 1]                                      # dA^L (D,N)
    return (T.astype(np.float16), E.astype(np.float16),
            Cb.astype(np.float16), P)


def _emit_kernel(nc, tile, mybir, make_identity):
    f16 = mybir.dt.float16
    f32 = mybir.dt.float32
    bf16 = mybir.dt.bfloat16

    onehotT = nc.dram_tensor("onehot_t", [VOCAB, BC * L], f16,
                             kind="ExternalInput").ap()
    embs = nc.dram_tensor("emb_s", [VOCAB, DPC], f16, kind="ExternalInput").ap()
    # [group, j, ch_in_group, i]
    t_all = nc.dram_tensor("t_all", [DPC // GT, L, GT, L], f16,
                           kind="ExternalInput").ap()
    e_all = nc.dram_tensor("e_all", [DPC // GE, L, GE, N_STATE], f16,
                           kind="ExternalInput").ap()
    # [group, p=(par,n), q_in_group, i]
    cb_all = nc.dram_tensor("cb_all", [64 // GQ, 128, GQ, L], f16,
                            kind="ExternalInput").ap()
    # Pm[p, q, 0]=0 (segmented-scan reset), Pm[p, q, c>=1] = dA^L
    pm = nc.dram_tensor("pm", [128, 64, NCH + 1], f16,
                        kind="ExternalInput").ap()
    w2 = nc.dram_tensor("w2", [DPC, VOCAB], f16, kind="ExternalInput").ap()
    out_t = nc.dram_tensor("out_t", [VOCAB, BC * L], bf16,
                           kind="ExternalOutput").ap()

    with tile.TileContext(nc) as tc, ExitStack() as ctx:
        persist = ctx.enter_context(tc.tile_pool(name="persist", bufs=1))
        # u and y share this buffer: u[j, b, c, d]; phase D overwrites
        # channel-pair slices with y[i, b, c, dl] after their last u read.
        uy = persist.tile([128, BATCH, NCH, DPC], f16, name="uy")
        # x_src: S[c-2] landing zone (cols 0,1 zero); x_sc: h_start after scan
        x_src = persist.tile([128, BATCH, 64, NCH + 1], f16, name="x_src")
        x_sc = persist.tile([128, BATCH, 64, NCH + 1], f16, name="x_sc")
        pm_sb = persist.tile([128, 64, NCH + 1], f16, name="pm_sb")
        emb_sb = persist.tile([VOCAB, DPC], f16, name="emb_sb")
        w2_sb = persist.tile([DPC, VOCAB], f16, name="w2_sb")
        ident = persist.tile([128, 128], f16, name="ident")

        make_identity(nc, ident)

        def cp(k, out, in_):
            # GPSIMD cannot read PSUM; alternate the two engines that can.
            if k % 2 == 0:
                nc.vector.tensor_copy(out, in_)
            else:
                nc.scalar.copy(out, in_)

        ncp = 0

        # ---- Phase A: embedding (one-hot @ emb slice) -> uy ----
        # Pool-queue order matters: first one-hot tile + emb before anything
        # else so the PE can start immediately; w2/pm/memset follow.
        with tc.tile_pool(name="ohp", bufs=4) as ohp, \
             tc.tile_pool(name="ps_a", bufs=3, space="PSUM") as ps_a:
            # first two groups are small so the PE starts sooner
            groups = [(0, 8), (8, 24)] + [(32 * g, 32) for g in range(1, 8)]
            for g, (t0, gl) in enumerate(groups):
                oh = ohp.tile([VOCAB, gl, L], f16)
                # alternate issue queues so neither sequencer gates phase A
                (nc.sync if g % 2 == 0 else nc.gpsimd).dma_start(
                    out=oh, in_=onehotT[:, t0 * L:(t0 + gl) * L])
                if g == 0:
                    nc.gpsimd.dma_start(out=emb_sb, in_=embs)
                elif g == 1:
                    nc.gpsimd.dma_start(out=pm_sb, in_=pm)
                    nc.gpsimd.memset(x_src[:, :, :, 0:2], 0.0)
                elif g == 2:
                    nc.gpsimd.dma_start(out=w2_sb, in_=w2)
                for i in range(gl):
                    t = t0 + i
                    b, c = t // NCH, t % NCH
                    if t % 8 == 0:              # 2-bank PSUM tile, 8 tiles
                        ups = ps_a.tile([128, 8, DPC], f32)
                    nc.tensor.matmul(ups[:, t % 8, :], lhsT=oh[:, i, :],
                                     rhs=emb_sb, start=True, stop=True)
                    if t % 8 == 7:
                        cp(ncp,
                           uy[:, b, c - 7:c + 1, :], ups)
                        ncp += 1

        # ---- Phase B + C: chunk-end states, with the segmented scan
        # interleaved per q-half so it hides under B's second half and
        # phase D's T-matmuls. ----
        twp = ctx.enter_context(tc.tile_pool(name="twp", bufs=6))
        cbp = ctx.enter_context(tc.tile_pool(name="cbp", bufs=3))
        bctx = ExitStack()
        ewp = bctx.enter_context(tc.tile_pool(name="ewp", bufs=8))
        ps_s = bctx.enter_context(tc.tile_pool(name="ps_s", bufs=3,
                                               space="PSUM"))
        ew_tiles = {}

        def b_dma(g):
            e_w = ewp.tile([L, GE, N_STATE], f16)
            nc.gpsimd.dma_start(out=e_w, in_=e_all[g])
            ew_tiles[g] = e_w

        def b_mms(g, k0, act_bias=False):
            e_w = ew_tiles[g]
            for i in range(GE // 4):            # 2 q (4 channels) per PSUM
                q0 = (g * GE) // 2 + 2 * i
                sp = ps_s.tile([128, 2, BATCH, NCH - 1], f32)
                for k in range(2):
                    for par in range(2):
                        dl = 2 * (q0 + k) + par
                        nc.tensor.matmul(
                            sp[64 * par:64 * (par + 1), k],
                            lhsT=e_w[:, 4 * i + 2 * k + par, :],
                            rhs=uy[:, :, 0:NCH - 1, dl],
                            start=True, stop=True,
                            tile_position=(0, 64 * par) if par else None)
                # while the DVE runs the half-scans, drain mostly via Act
                kk = 1 if act_bias else k0 + i
                cp(kk, x_src[:, :, q0:q0 + 2, 2:NCH + 1],
                   sp.transpose([0, 2, 1, 3]))
            return k0 + GE // 4

        def quarter_scans(k):
            # walrus: TensorScalarPtr is DVE-only. Quarter k covers q
            # 16k..16k+15 and only needs e-groups 2k and 2k+1 done, so each
            # quarter is emitted as early as its data allows and finishes
            # well before phase D's Cb matmuls reach its q range.
            qs = slice(16 * k, 16 * (k + 1))
            for b in range(BATCH):
                nc.vector.tensor_tensor_scan(
                    out=x_sc[:, b, qs].opt(), data0=pm_sb[:, qs].opt(),
                    data1=x_src[:, b, qs].opt(), initial=0.0,
                    op0=mybir.AluOpType.mult, op1=mybir.AluOpType.add)

        for g in range(8):                      # prefetch all E weights
            b_dma(g)
        for g in range(8):
            ncp = b_mms(g, ncp, act_bias=(g >= 2))
            if g % 2 == 1:
                quarter_scans(g // 2)
        # prefetch D weights for q<24 ahead of phase D
        tws, cbs = [], []
        for g in range(6):
            t_w = twp.tile([L, GT, L], f16)
            nc.gpsimd.dma_start(out=t_w, in_=t_all[g])
            tws.append(t_w)
        for g in range(3):
            cb_w = cbp.tile([128, GQ, L], f16)
            nc.gpsimd.dma_start(out=cb_w, in_=cb_all[g])
            cbs.append(cb_w)
        bctx.close()                            # release ewp + ps_s

        # ---- Phase D: y = T^T u (+) Cb^T h_start, overwrites uy ----
        with tc.tile_pool(name="ps_y", bufs=6, space="PSUM") as ps_y:
            for q in range(64):
                if q % (GT // 2) == 0:
                    if q < 24:
                        t_w = tws[q // 4]
                    else:
                        t_w = twp.tile([L, GT, L], f16)
                        nc.gpsimd.dma_start(out=t_w, in_=t_all[(2 * q) // GT])
                if q % GQ == 0:
                    if q < 24:
                        cb_w = cbs[q // 8]
                    else:
                        cb_w = cbp.tile([128, GQ, L], f16)
                        nc.gpsimd.dma_start(out=cb_w, in_=cb_all[q // GQ])
                yp = ps_y.tile([128, 2, BATCH, NCH], f32)
                for par in range(2):
                    # open+close one accumulation group per bank half —
                    # two simultaneously-open groups in one PSUM zero
                    # region are illegal.
                    dl = 2 * q + par
                    nc.tensor.matmul(yp[:, par], lhsT=t_w[:, dl % GT, :],
                                     rhs=uy[:, :, :, dl],
                                     start=True, stop=False)
                    nc.tensor.matmul(
                        yp[:, par],
                        lhsT=cb_w[64 * par:64 * (par + 1), q % GQ, :],
                        rhs=x_sc[64 * par:64 * (par + 1), :, q, 1:NCH + 1],
                        start=False, stop=True,
                        tile_position=(64 * par, 0) if par else None)
                # 3:5 DVE:Act split — the DVE still owes the h1 scans here
                cp(0 if q % 8 >= 5 else 1, uy[:, :, :, 2 * q:2 * q + 2],
                   yp.transpose([0, 2, 3, 1]))
                ncp += 1

        # ---- Phase E: transpose y + output projection -> out_t ----
        # Even G: PE transpose via PSUM; odd G: XBAR DMA transpose on the
        # otherwise-idle SP sequencer (SBUF->SBUF, no PSUM drain).
        with tc.tile_pool(name="ytp", bufs=4) as ytp, \
             tc.tile_pool(name="lop", bufs=3) as lop, \
             tc.tile_pool(name="ps_t", bufs=3, space="PSUM") as ps_t, \
             tc.tile_pool(name="ps_o", bufs=2, space="PSUM") as ps_o:
            for G in range(32):                 # 8 t-tiles per group
                yt = ytp.tile([128, 8, 128], f16)
                tp = ps_t.tile([128, 8, 128], f16)
                for k in range(8):
                    t = 8 * G + k
                    b, c = t // NCH, t % NCH
                    nc.tensor.transpose(tp[:, k, :], uy[:, b, c, :], ident)
                cp(ncp, yt, tp)
                ncp += 1
                # v-stacked projection: even G -> PSUM rows 0:64, odd G ->
                # rows 64:128 (weights loaded at PE columns 64:128), so one
                # [128, 1024] copy drains two G-groups of logits.
                if G % 4 == 0:
                    lo = lop.tile([128, 2, 8, L], bf16)   # 32 t
                if G % 2 == 0:
                    po = ps_o.tile([128, 2, 4 * L], f32)  # 2 banks
                sub = G % 2
                for h in range(2):
                    nc.tensor.matmul(po[64 * sub:64 * (sub + 1), h],
                                     lhsT=w2_sb,
                                     rhs=yt[:, 4 * h:4 * h + 4, :],
                                     start=True, stop=True,
                                     tile_position=(0, 64 * sub) if sub
                                     else None)
                if G % 2 == 1:
                    cp(ncp, lo[:, (G // 2) % 2], po)
                    ncp += 1
                if G % 4 == 3:
                    m2 = G // 4
                    # out col = v*32768 + 4096*m2 + 2048*mm + 1024*sub + f
                    full = out_t[:, m2 * 32 * L:(m2 + 1) * 32 * L] \
                        .rearrange("v (mm s f) -> v mm s f", mm=2, s=2)
                    for sub in range(2):
                        eng = nc.sync if sub == 0 else nc.gpsimd
                        eng.dma_start(
                            out=full[:, :, sub, :],
                            in_=lo[64 * sub:64 * (sub + 1)].opt())


def _build_nc():
    import concourse.tile as tile
    from concourse import bacc, mybir
    from concourse.masks import make_identity

    nc = bacc.Bacc(trn_type="TRN2", target_bir_lowering=False, debug=False)
    _emit_kernel(nc, tile, mybir, make_identity)
    nc.compile()
    return nc


_NC_CACHE = None


def _make_in_maps(x, emb, log_neg_A, B, C, D, log_dt, W_out, b_out):
    x = np.asarray(x).astype(np.int64)
    emb = np.asarray(emb, np.float32)
    log_neg_A = np.asarray(log_neg_A, np.float32)
    B_in = np.asarray(B, np.float32)
    C = np.asarray(C, np.float32)
    D_in = np.asarray(D, np.float32)
    log_dt = np.asarray(log_dt, np.float32)
    W_out = np.asarray(W_out, np.float32)

    T, E, Cb, P = _precompute_host(emb, log_neg_A, B_in, C, D_in, log_dt, W_out)

    # one-hot, token order col = (b*NCH + c)*L + j
    toks = x.reshape(BATCH, NCH, L).reshape(-1)
    onehotT = (np.arange(VOCAB)[:, None] == toks[None, :]).astype(np.float16)

    in_maps = []
    for core in range(NCORES):
        ds = slice(core * DPC, (core + 1) * DPC)
        # Pm layout [p=(par,n), q, c']: p = 64*par + n, d = 2*q + par
        Pc = P[ds].reshape(64, 2, N_STATE).transpose(1, 2, 0).reshape(128, 64)
        Pm = np.zeros((128, 64, NCH + 1), np.float16)
        Pm[:, :, 1:] = Pc[:, :, None].astype(np.float16)
        # t_all: [DPC,L,L] -> [DPC/GT, L, GT, L]
        Tc = np.ascontiguousarray(
            T[ds].reshape(DPC // GT, GT, L, L).transpose(0, 2, 1, 3))
        # e_all: [DPC,L,N] -> [DPC/GE, L, GE, N]
        Ec = np.ascontiguousarray(
            E[ds].reshape(DPC // GE, GE, L, N_STATE).transpose(0, 2, 1, 3))
        # cb_all: [DPC,N,L] -> pair-pack [64, 128=(par,n), L] -> groups of GQ
        Cbp = Cb[ds].reshape(64, 2 * N_STATE, L)   # [q, (par,n), L]
        Cbc = np.ascontiguousarray(
            Cbp.reshape(64 // GQ, GQ, 128, L).transpose(0, 2, 1, 3))
        in_maps.append({
            "onehot_t": onehotT,
            "emb_s": np.ascontiguousarray(emb[:, ds]).astype(np.float16),
            "t_all": Tc,
            "e_all": Ec,
            "cb_all": Cbc,
            "pm": Pm,
            "w2": np.ascontiguousarray(W_out[ds]).astype(np.float16),
        })
    return in_maps


def _postprocess(results, b_out):
    logitsT = np.zeros((VOCAB, BC * L), np.float64)
    for r in results:
        logitsT += r["out_t"].astype(np.float64)
    # col = (b*NCH + c)*L + j
    out = logitsT.T.reshape(BATCH, SEQ, VOCAB)
    return (out + np.asarray(b_out).astype(np.float64)).astype(np.float32)


def kernel(x, emb, log_neg_A, B, C, D, log_dt, W_out, b_out):
    global LAST_RESULTS, _NC_CACHE
    from concourse.bass_utils import run_bass_kernel_spmd

    in_maps = _make_in_maps(x, emb, log_neg_A, B, C, D, log_dt, W_out, b_out)

    if _NC_CACHE is None:
        _NC_CACHE = _build_nc()
    nc = _NC_CACHE

    trace = bool(int(os.environ.get("BASS_TRACE", "0") or "0"))
    LAST_RESULTS = run_bass_kernel_spmd(
        nc, in_maps, core_ids=list(range(NCORES)), trace=trace)

    return _postprocess(LAST_RESULTS.results, b_out)


# revision 66
# speedup vs baseline: 1.0628x; 1.0559x over previous
"""S4D "CopyingModel" Trainium2 Bass kernel (V2).

Math: logits = (s4d_scan(emb[x]) + emb[x]*D) @ W_out + b_out, with a
per-channel diagonal SSM (d_model=1024 channels, d_state=64).

Strategy (8 NeuronCores, channel-sharded: 128 channels per core, every core
sees all 8 batches x 4096 tokens):
  - host precomputes (f64) discretized per-channel operators, fp16 on device:
      * T[d]  [L,L]  lower-tri Toeplitz of the conv kernel K (D skip folded)
      * E[d]  [L,N]  chunk-end state accumulator dA^(L-1-j)*dB
      * Cb[d] [N,L]  state->output C*dA^(i+1)
      * Pm    [p,q,33] chunk decay dA^L with a 0 reset column (segmented scan)
  - token/tile index t = b*32 + c (batch-major, chunk inner), chunk L=128
  - embedding gather = one-hot matmul; u and y share one SBUF buffer
    (y overwrites u channel pairs as phase D drains them)
  - chunk-carry scan = 8 DVE tensor_tensor_scan ops (segmented via Pm zeros)
  - output projection from PE-transposed y tiles; partials written as bf16;
    host sums cores + bias.

vs the original baseline: PSUM drains batched to full banks (4-8x fewer
DVE/Act copies), the 62-op elementwise chunk scan replaced by segmented
tensor_tensor_scan instructions split per (batch, q-half) and interleaved
with phase B so they hide under B's tail and D's T-matmuls, DMA issue
spread across SP (HWDGE) and Pool (SWDGE) queues with weight prefetch
emitted ahead of the Pool-queue scans, v-stacked output projection (two
64-row logit tiles per 128-partition PSUM bank via tile_position), u/y
sharing one SBUF buffer, and bf16 output partials (half the out DMA).
"""

import os
from contextlib import ExitStack

import numpy as np

BATCH = 8
SEQ = 4096
D_MODEL = 1024
N_STATE = 64
VOCAB = 64
L = 128                   # chunk length
NCH = SEQ // L            # 32 chunks
NCORES = 8
DPC = D_MODEL // NCORES   # 128 channels per core
BC = NCH * BATCH          # 256 (batch, chunk) tiles; t = b*NCH + c

GT = 8    # T channels per DMA group
GE = 16   # E channels per DMA group
GQ = 8    # Cb channel-pairs per DMA group

LAST_RESULTS = None       # BassKernelResults of the most recent run (for test.py)


def _precompute_host(emb, log_neg_A, Bmat, C, Dvec, log_dt, W_out):
    """Float64 host precompute of all device operands."""
    dt = np.exp(log_dt.astype(np.float64))                    # (D,)
    A = -np.exp(log_neg_A.astype(np.float64))                 # (D,N)
    dA = np.exp(dt[:, None] * A)                              # (D,N)
    dB = (dA - 1.0) / A * Bmat.astype(np.float64)             # (D,N)
    w = C.astype(np.float64) * dB                             # (D,N)

    # dApow[d,n,k] = dA^k, k=0..L-1
    dApow = np.ones((D_MODEL, N_STATE, L))
    np.cumprod(np.broadcast_to(dA[:, :, None], (D_MODEL, N_STATE, L - 1)),
               axis=2, out=dApow[:, :, 1:])
    K = np.einsum("dn,dnk->dk", w, dApow)                     # (D,L)
    K[:, 0] += Dvec.astype(np.float64)                        # fold skip

    # Toeplitz lhsT: T[d][j,i] = K[d, i-j] for i>=j
    T = np.zeros((D_MODEL, L, L), np.float32)
    Kf = K.astype(np.float32)
    for k in range(L):
        idx = np.arange(L - k)
        T[:, idx, idx + k] = Kf[:, k][:, None]

    # E lhsT [d, j, n] = dA^(L-1-j) * dB
    E = (dApow[:, :, ::-1] * dB[:, :, None]).transpose(0, 2, 1)  # (D,L,N)
    # Cb lhsT [d, n, i] = C * dA^(i+1)
    dApow1 = dApow * dA[:, :, None]
    Cb = C.astype(np.float64)[:, :, None] * dApow1               # (D,N,L)
    P = dApow1[:, :, L - 1]                                      # dA^L (D,N)
    return (T.astype(np.float16), E.astype(np.float16),
            Cb.astype(np.float16), P)


def _emit_kernel(nc, tile, mybir, make_identity):
    f16 = mybir.dt.float16
    f32 = mybir.dt.float32
    bf16 = mybir.dt.bfloat16

    onehotT = nc.dram_tensor("onehot_t", [VOCAB, BC * L], f16,
                             kind="ExternalInput").ap()
    embs = nc.dram_tensor("emb_s", [VOCAB, DPC], f16, kind="ExternalInput").ap()
    # [group, j, ch_in_group, i]
    t_all = nc.dram_tensor("t_all", [DPC // GT, L, GT, L], f16,
                           kind="ExternalInput").ap()
    e_all = nc.dram_tensor("e_all", [DPC // GE, L, GE, N_STATE], f16,
                           kind="ExternalInput").ap()
    # [group, p=(par,n), q_in_group, i]
    cb_all = nc.dram_tensor("cb_all", [64 // GQ, 128, GQ, L], f16,
                            kind="ExternalInput").ap()
    # Pm[p, q, 0]=0 (segmented-scan reset), Pm[p, q, c>=1] = dA^L
    pm = nc.dram_tensor("pm", [128, 64, NCH + 1], f16,
                        kind="ExternalInput").ap()
    w2 = nc.dram_tensor("w2", [DPC, VOCAB], f16, kind="ExternalInput").ap()
    out_t = nc.dram_tensor("out_t", [VOCAB, BC * L], bf16,
                           kind="ExternalOutput").ap()

    with tile.TileContext(nc) as tc, ExitStack() as ctx:
        persist = ctx.enter_context(tc.tile_pool(name="persist", bufs=1))
        # u and y share this buffer: u[j, b, c, d]; phase D overwrites
        # channel-pair slices with y[i, b, c, dl] after their last u read.
        uy = persist.tile([128, BATCH, NCH, DPC], f16, name="uy")
        # x_src: S[c-2] landing zone (cols 0,1 zero); x_sc: h_start after scan
        x_src = persist.tile([128, BATCH, 64, NCH + 1], f16, name="x_src")
        x_sc = persist.tile([128, BATCH, 64, NCH + 1], f16, name="x_sc")
        pm_sb = persist.tile([128, 64, NCH + 1], f16, name="pm_sb")
        emb_sb = persist.tile([VOCAB, DPC], f16, name="emb_sb")
        w2_sb = persist.tile([DPC, VOCAB], f16, name="w2_sb")
        ident = persist.tile([128, 128], f16, name="ident")

        make_identity(nc, ident)

        def cp(k, out, in_):
            # GPSIMD cannot read PSUM; alternate the two engines that can.
            if k % 2 == 0:
                nc.vector.tensor_copy(out, in_)
            else:
                nc.scalar.copy(out, in_)

        ncp = 0

        # ---- Phase A: embedding (one-hot @ emb slice) -> uy ----
        # Pool-queue order matters: first one-hot tile + emb before anything
        # else so the PE can start immediately; w2/pm/memset follow.
        with tc.tile_pool(name="ohp", bufs=4) as ohp, \
             tc.tile_pool(name="ps_a", bufs=3, space="PSUM") as ps_a:
            # first two groups are small so the PE starts sooner
            groups = [(0, 8), (8, 24)] + [(32 * g, 32) for g in range(1, 8)]
            for g, (t0, gl) in enumerate(groups):
                oh = ohp.tile([VOCAB, gl, L], f16)
                # alternate issue queues so neither sequencer gates phase A
                (nc.sync if g % 2 == 0 else nc.gpsimd).dma_start(
                    out=oh, in_=onehotT[:, t0 * L:(t0 + gl) * L])
                if g == 0:
                    nc.gpsimd.dma_start(out=emb_sb, in_=embs)
                elif g == 1:
                    nc.gpsimd.dma_start(out=pm_sb, in_=pm)
                    nc.gpsimd.memset(x_src[:, :, :, 0:2], 0.0)
                elif g == 2:
                    nc.gpsimd.dma_start(out=w2_sb, in_=w2)
                for i in range(gl):
                    t = t0 + i
                    b, c = t // NCH, t % NCH
                    if t % 8 == 0:              # 2-bank PSUM tile, 8 tiles
                        ups = ps_a.tile([128, 8, DPC], f32)
                    nc.tensor.matmul(ups[:, t % 8, :], lhsT=oh[:, i, :],
                                     rhs=emb_sb, start=True, stop=True)
                    if t % 8 == 7:
                        cp(ncp,
                           uy[:, b, c - 7:c + 1, :], ups)
                        ncp += 1

        # ---- Phase B + C: chunk-end states, with the segmented scan
        # interleaved per q-half so it hides under B's second half and
        # phase D's T-matmuls. ----
        twp = ctx.enter_context(tc.tile_pool(name="twp", bufs=6))
        cbp = ctx.enter_context(tc.tile_pool(name="cbp", bufs=3))
        bctx = ExitStack()
        ewp = bctx.enter_context(tc.tile_pool(name="ewp", bufs=8))
        ps_s = bctx.enter_context(tc.tile_pool(name="ps_s", bufs=3,
                                               space="PSUM"))
        ew_tiles = {}

        def b_dma(g):
            e_w = ewp.tile([L, GE, N_STATE], f16)
            nc.gpsimd.dma_start(out=e_w, in_=e_all[g])
            ew_tiles[g] = e_w

        def b_mms(g, k0, act_bias=False):
            e_w = ew_tiles[g]
            for i in range(GE // 4):            # 2 q (4 channels) per PSUM
                q0 = (g * GE) // 2 + 2 * i
                sp = ps_s.tile([128, 2, BATCH, NCH - 1], f32)
                for k in range(2):
                    for par in range(2):
                        dl = 2 * (q0 + k) + par
                        nc.tensor.matmul(
                            sp[64 * par:64 * (par + 1), k],
                            lhsT=e_w[:, 4 * i + 2 * k + par, :],
                            rhs=uy[:, :, 0:NCH - 1, dl],
                            start=True, stop=True,
                            tile_position=(0, 64 * par) if par else None)
                # while the DVE runs the half-scans, drain mostly via Act
                kk = 1 if act_bias else k0 + i
                cp(kk, x_src[:, :, q0:q0 + 2, 2:NCH + 1],
                   sp.transpose([0, 2, 1, 3]))
            return k0 + GE // 4

        def quarter_scans(k):
            # walrus: TensorScalarPtr is DVE-only. Quarter k covers q
            # 16k..16k+15 and only needs e-groups 2k and 2k+1 done, so each
            # quarter is emitted as early as its data allows and finishes
            # well before phase D's Cb matmuls reach its q range.
            qs = slice(16 * k, 16 * (k + 1))
            for b in range(BATCH):
                nc.vector.tensor_tensor_scan(
                    out=x_sc[:, b, qs].opt(), data0=pm_sb[:, qs].opt(),
                    data1=x_src[:, b, qs].opt(), initial=0.0,
                    op0=mybir.AluOpType.mult, op1=mybir.AluOpType.add)

        for g in range(8):                      # prefetch all E weights
            b_dma(g)
        for g in range(8):
            ncp = b_mms(g, ncp, act_bias=(g >= 2))
            if g % 2 == 1:
                quarter_scans(g // 2)
        # prefetch D weights for q<24 ahead of phase D
        tws, cbs = [], []
        for g in range(6):
            t_w = twp.tile([L, GT, L], f16)
            nc.gpsimd.dma_start(out=t_w, in_=t_all[g])
            tws.append(t_w)
        for g in range(3):
            cb_w = cbp.tile([128, GQ, L], f16)
            nc.gpsimd.dma_start(out=cb_w, in_=cb_all[g])
            cbs.append(cb_w)
        bctx.close()                            # release ewp + ps_s

        # ---- Phase D: y = T^T u (+) Cb^T h_start, overwrites uy ----
        with tc.tile_pool(name="ps_y", bufs=8, space="PSUM") as ps_y:
            for q in range(64):
                if q % (GT // 2) == 0:
                    if q < 24:
                        t_w = tws[q // 4]
                    else:
                        t_w = twp.tile([L, GT, L], f16)
                        nc.gpsimd.dma_start(out=t_w, in_=t_all[(2 * q) // GT])
                if q % GQ == 0:
                    if q < 24:
                        cb_w = cbs[q // 8]
                    else:
                        cb_w = cbp.tile([128, GQ, L], f16)
                        nc.gpsimd.dma_start(out=cb_w, in_=cb_all[q // GQ])
                yp = ps_y.tile([128, 2, BATCH, NCH], f32)
                for par in range(2):
                    # open+close one accumulation group per bank half —
                    # two simultaneously-open groups in one PSUM zero
                    # region are illegal.
                    dl = 2 * q + par
                    nc.tensor.matmul(yp[:, par], lhsT=t_w[:, dl % GT, :],
                                     rhs=uy[:, :, :, dl],
                                     start=True, stop=False)
                    nc.tensor.matmul(
                        yp[:, par],
                        lhsT=cb_w[64 * par:64 * (par + 1), q % GQ, :],
                        rhs=x_sc[64 * par:64 * (par + 1), :, q, 1:NCH + 1],
                        start=False, stop=True,
                        tile_position=(64 * par, 0) if par else None)
                # 3:5 DVE:Act split — the DVE still owes the h1 scans here
                cp(0 if q % 8 >= 5 else 1, uy[:, :, :, 2 * q:2 * q + 2],
                   yp.transpose([0, 2, 3, 1]))
                ncp += 1

        # ---- Phase E: transpose y + output projection -> out_t ----
        # Even G: PE transpose via PSUM; odd G: XBAR DMA transpose on the
        # otherwise-idle SP sequencer (SBUF->SBUF, no PSUM drain).
        with tc.tile_pool(name="ytp", bufs=4) as ytp, \
             tc.tile_pool(name="lop", bufs=3) as lop, \
             tc.tile_pool(name="ps_t", bufs=3, space="PSUM") as ps_t, \
             tc.tile_pool(name="ps_o", bufs=2, space="PSUM") as ps_o:
            for G in range(32):                 # 8 t-tiles per group
                yt = ytp.tile([128, 8, 128], f16)
                tp = ps_t.tile([128, 8, 128], f16)
                for k in range(8):
                    t = 8 * G + k
                    b, c = t // NCH, t % NCH
                    nc.tensor.transpose(tp[:, k, :], uy[:, b, c, :], ident)
                cp(0, yt, tp)
                ncp += 1
                # v-stacked projection: even G -> PSUM rows 0:64, odd G ->
                # rows 64:128 (weights loaded at PE columns 64:128), so one
                # [128, 1024] copy drains two G-groups of logits.
                if G % 4 == 0:
                    lo = lop.tile([128, 2, 8, L], bf16)   # 32 t
                if G % 2 == 0:
                    po = ps_o.tile([128, 2, 4 * L], f32)  # 2 banks
                sub = G % 2
                for h in range(2):
                    nc.tensor.matmul(po[64 * sub:64 * (sub + 1), h],
                                     lhsT=w2_sb,
                                     rhs=yt[:, 4 * h:4 * h + 4, :],
                                     start=True, stop=True,
                                     tile_position=(0, 64 * sub) if sub
                                     else None)
                if G % 2 == 1:
                    cp(1, lo[:, (G // 2) % 2], po)
                    ncp += 1
                if G % 4 == 3:
                    m2 = G // 4
                    # out col = v*32768 + 4096*m2 + 2048*mm + 1024*sub + f
                    full = out_t[:, m2 * 32 * L:(m2 + 1) * 32 * L] \
                        .rearrange("v (mm s f) -> v mm s f", mm=2, s=2)
                    for sub in range(2):
                        eng = nc.sync if sub == 0 else nc.gpsimd
                        eng.dma_start(
                            out=full[:, :, sub, :],
                            in_=lo[64 * sub:64 * (sub + 1)].opt())


def _build_nc():
    import concourse.tile as tile
    from concourse import bacc, mybir
    from concourse.masks import make_identity

    nc = bacc.Bacc(trn_type="TRN2", target_bir_lowering=False, debug=False)
    _emit_kernel(nc, tile, mybir, make_identity)
    nc.compile()
    return nc


_NC_CACHE = None


def _make_in_maps(x, emb, log_neg_A, B, C, D, log_dt, W_out, b_out):
    x = np.asarray(x).astype(np.int64)
    emb = np.asarray(emb, np.float32)
    log_neg_A = np.asarray(log_neg_A, np.float32)
    B_in = np.asarray(B, np.float32)
    C = np.asarray(C, np.float32)
    D_in = np.asarray(D, np.float32)
    log_dt = np.asarray(log_dt, np.float32)
    W_out = np.asarray(W_out, np.float32)

    T, E, Cb, P = _precompute_host(emb, log_neg_A, B_in, C, D_in, log_dt, W_out)

    # one-hot, token order col = (b*NCH + c)*L + j
    toks = x.reshape(BATCH, NCH, L).reshape(-1)
    onehotT = (np.arange(VOCAB)[:, None] == toks[None, :]).astype(np.float16)

    in_maps = []
    for core in range(NCORES):
        ds = slice(core * DPC, (core + 1) * DPC)
        # Pm layout [p=(par,n), q, c']: p = 64*par + n, d = 2*q + par
        Pc = P[ds].reshape(64, 2, N_STATE).transpose(1, 2, 0).reshape(128, 64)
        Pm = np.zeros((128, 64, NCH + 1), np.float16)
        Pm[:, :, 1:] = Pc[:, :, None].astype(np.float16)
        # t_all: [DPC,L,L] -> [DPC/GT, L, GT, L]
        Tc = np.ascontiguousarray(
            T[ds].reshape(DPC // GT, GT, L, L).transpose(0, 2, 1, 3))
        # e_all: [DPC,L,N] -> [DPC/GE, L, GE, N]
        Ec = np.ascontiguousarray(
            E[ds].reshape(DPC // GE, GE, L, N_STATE).transpose(0, 2, 1, 3))
        # cb_all: [DPC,N,L] -> pair-pack [64, 128=(par,n), L] -> groups of GQ
        Cbp = Cb[ds].reshape(64, 2 * N_STATE, L)   # [q, (par,n), L]
        Cbc = np.ascontiguousarray(
            Cbp.reshape(64 // GQ, GQ, 128, L).transpose(0, 2, 1, 3))
        in_maps.append({
            "onehot_t": onehotT,
            "emb_s": np.ascontiguousarray(emb[:, ds]).astype(np.float16),
            "t_all": Tc,
            "e_all": Ec,
            "cb_all": Cbc,
            "pm": Pm,
            "w2": np.ascontiguousarray(W_out[ds]).astype(np.float16),
        })
    return in_maps


def _postprocess(results, b_out):
    logitsT = np.zeros((VOCAB, BC * L), np.float64)
    for r in results:
        logitsT += r["out_t"].astype(np.float64)
    # col = (b*NCH + c)*L + j
    out = logitsT.T.reshape(BATCH, SEQ, VOCAB)
    return (out + np.asarray(b_out).astype(np.float64)).astype(np.float32)


def kernel(x, emb, log_neg_A, B, C, D, log_dt, W_out, b_out):
    global LAST_RESULTS, _NC_CACHE
    from concourse.bass_utils import run_bass_kernel_spmd

    in_maps = _make_in_maps(x, emb, log_neg_A, B, C, D, log_dt, W_out, b_out)

    if _NC_CACHE is None:
        _NC_CACHE = _build_nc()
    nc = _NC_CACHE

    trace = bool(int(os.environ.get("BASS_TRACE", "0") or "0"))
    LAST_RESULTS = run_bass_kernel_spmd(
        nc, in_maps, core_ids=list(range(NCORES)), trace=trace)

    return _postprocess(LAST_RESULTS.results, b_out)


# revision 70
# speedup vs baseline: 1.0707x; 1.0074x over previous
"""S4D "CopyingModel" Trainium2 Bass kernel (V2).

Math: logits = (s4d_scan(emb[x]) + emb[x]*D) @ W_out + b_out, with a
per-channel diagonal SSM (d_model=1024 channels, d_state=64).

Strategy (8 NeuronCores, channel-sharded: 128 channels per core, every core
sees all 8 batches x 4096 tokens):
  - host precomputes (f64) discretized per-channel operators, fp16 on device:
      * T[d]  [L,L]  lower-tri Toeplitz of the conv kernel K (D skip folded)
      * E[d]  [L,N]  chunk-end state accumulator dA^(L-1-j)*dB
      * Cb[d] [N,L]  state->output C*dA^(i+1)
      * Pm    [p,q,33] chunk decay dA^L with a 0 reset column (segmented scan)
  - token/tile index t = b*32 + c (batch-major, chunk inner), chunk L=128
  - embedding gather = one-hot matmul; u and y share one SBUF buffer
    (y overwrites u channel pairs as phase D drains them)
  - chunk-carry scan = 8 DVE tensor_tensor_scan ops (segmented via Pm zeros)
  - output projection from PE-transposed y tiles; partials written as bf16;
    host sums cores + bias.

vs the original baseline: PSUM drains batched to full banks (4-8x fewer
DVE/Act copies), the 62-op elementwise chunk scan replaced by segmented
tensor_tensor_scan instructions split per (batch, q-half) and interleaved
with phase B so they hide under B's tail and D's T-matmuls, DMA issue
spread across SP (HWDGE) and Pool (SWDGE) queues with weight prefetch
emitted ahead of the Pool-queue scans, v-stacked output projection (two
64-row logit tiles per 128-partition PSUM bank via tile_position), u/y
sharing one SBUF buffer, and bf16 output partials (half the out DMA).
"""

import os
from contextlib import ExitStack

import numpy as np

BATCH = 8
SEQ = 4096
D_MODEL = 1024
N_STATE = 64
VOCAB = 64
L = 128                   # chunk length
NCH = SEQ // L            # 32 chunks
NCORES = 8
DPC = D_MODEL // NCORES   # 128 channels per core
BC = NCH * BATCH          # 256 (batch, chunk) tiles; t = b*NCH + c

GT = 8    # T channels per DMA group
GE = 16   # E channels per DMA group
GQ = 8    # Cb channel-pairs per DMA group

LAST_RESULTS = None       # BassKernelResults of the most recent run (for test.py)


def _precompute_host(emb, log_neg_A, Bmat, C, Dvec, log_dt, W_out):
    """Float64 host precompute of all device operands."""
    dt = np.exp(log_dt.astype(np.float64))                    # (D,)
    A = -np.exp(log_neg_A.astype(np.float64))                 # (D,N)
    dA = np.exp(dt[:, None] * A)                              # (D,N)
    dB = (dA - 1.0) / A * Bmat.astype(np.float64)             # (D,N)
    w = C.astype(np.float64) * dB                             # (D,N)

    # dApow[d,n,k] = dA^k, k=0..L-1
    dApow = np.ones((D_MODEL, N_STATE, L))
    np.cumprod(np.broadcast_to(dA[:, :, None], (D_MODEL, N_STATE, L - 1)),
               axis=2, out=dApow[:, :, 1:])
    K = np.einsum("dn,dnk->dk", w, dApow)                     # (D,L)
    K[:, 0] += Dvec.astype(np.float64)                        # fold skip

    # Toeplitz lhsT: T[d][j,i] = K[d, i-j] for i>=j
    T = np.zeros((D_MODEL, L, L), np.float32)
    Kf = K.astype(np.float32)
    for k in range(L):
        idx = np.arange(L - k)
        T[:, idx, idx + k] = Kf[:, k][:, None]

    # E lhsT [d, j, n] = dA^(L-1-j) * dB
    E = (dApow[:, :, ::-1] * dB[:, :, None]).transpose(0, 2, 1)  # (D,L,N)
    # Cb lhsT [d, n, i] = C * dA^(i+1)
    dApow1 = dApow * dA[:, :, None]
    Cb = C.astype(np.float64)[:, :, None] * dApow1               # (D,N,L)
    P = dApow1[:, :, L - 1]                                      # dA^L (D,N)
    return (T.astype(np.float16), E.astype(np.float16),
            Cb.astype(np.float16), P)


def _emit_kernel(nc, tile, mybir, make_identity):
    f16 = mybir.dt.float16
    f32 = mybir.dt.float32
    bf16 = mybir.dt.bfloat16

    onehotT = nc.dram_tensor("onehot_t", [VOCAB, BC * L], f16,
                             kind="ExternalInput").ap()
    embs = nc.dram_tensor("emb_s", [VOCAB, DPC], f16, kind="ExternalInput").ap()
    # [group, j, ch_in_group, i]
    t_all = nc.dram_tensor("t_all", [DPC // GT, L, GT, L], f16,
                           kind="ExternalInput").ap()
    e_all = nc.dram_tensor("e_all", [DPC // GE, L, GE, N_STATE], f16,
                           kind="ExternalInput").ap()
    # [group, p=(par,n), q_in_group, i]
    cb_all = nc.dram_tensor("cb_all", [64 // GQ, 128, GQ, L], f16,
                            kind="ExternalInput").ap()
    # Pm[p, q, 0]=0 (segmented-scan reset), Pm[p, q, c>=1] = dA^L
    pm = nc.dram_tensor("pm", [128, 64, NCH + 1], f16,
                        kind="ExternalInput").ap()
    w2 = nc.dram_tensor("w2", [DPC, VOCAB], f16, kind="ExternalInput").ap()
    out_t = nc.dram_tensor("out_t", [VOCAB, BC * L], bf16,
                           kind="ExternalOutput").ap()

    with tile.TileContext(nc) as tc, ExitStack() as ctx:
        persist = ctx.enter_context(tc.tile_pool(name="persist", bufs=1))
        # u and y share this buffer: u[j, b, c, d]; phase D overwrites
        # channel-pair slices with y[i, b, c, dl] after their last u read.
        uy = persist.tile([128, BATCH, NCH, DPC], f16, name="uy")
        # x_src: S[c-2] landing zone (cols 0,1 zero); x_sc: h_start after scan
        x_src = persist.tile([128, BATCH, 64, NCH + 1], f16, name="x_src")
        x_sc = persist.tile([128, BATCH, 64, NCH + 1], f16, name="x_sc")
        pm_sb = persist.tile([128, 64, NCH + 1], f16, name="pm_sb")
        emb_sb = persist.tile([VOCAB, DPC], f16, name="emb_sb")
        w2_sb = persist.tile([DPC, VOCAB], f16, name="w2_sb")
        ident = persist.tile([128, 128], f16, name="ident")

        make_identity(nc, ident)

        def cp(k, out, in_):
            # GPSIMD cannot read PSUM; alternate the two engines that can.
            if k % 2 == 0:
                nc.vector.tensor_copy(out, in_)
            else:
                nc.scalar.copy(out, in_)

        ncp = 0

        # ---- Phase A: embedding (one-hot @ emb slice) -> uy ----
        # Pool-queue order matters: first one-hot tile + emb before anything
        # else so the PE can start immediately; w2/pm/memset follow.
        with tc.tile_pool(name="ohp", bufs=4) as ohp, \
             tc.tile_pool(name="ps_a", bufs=3, space="PSUM") as ps_a:
            # first two groups are small so the PE starts sooner
            groups = [(0, 8), (8, 24)] + [(32 * g, 32) for g in range(1, 8)]
            for g, (t0, gl) in enumerate(groups):
                oh = ohp.tile([VOCAB, gl, L], f16)
                # alternate issue queues so neither sequencer gates phase A
                (nc.sync if g % 2 == 0 else nc.gpsimd).dma_start(
                    out=oh, in_=onehotT[:, t0 * L:(t0 + gl) * L])
                if g == 0:
                    nc.gpsimd.dma_start(out=emb_sb, in_=embs)
                elif g == 1:
                    nc.gpsimd.dma_start(out=pm_sb, in_=pm)
                    nc.gpsimd.memset(x_src[:, :, :, 0:2], 0.0)
                elif g == 2:
                    nc.gpsimd.dma_start(out=w2_sb, in_=w2)
                for i in range(gl):
                    t = t0 + i
                    b, c = t // NCH, t % NCH
                    if t % 8 == 0:              # 2-bank PSUM tile, 8 tiles
                        ups = ps_a.tile([128, 8, DPC], f32)
                    nc.tensor.matmul(ups[:, t % 8, :], lhsT=oh[:, i, :],
                                     rhs=emb_sb, start=True, stop=True)
                    if t % 8 == 7:
                        cp(ncp,
                           uy[:, b, c - 7:c + 1, :], ups)
                        ncp += 1

        # ---- Phase B + C: chunk-end states, with the segmented scan
        # interleaved per q-half so it hides under B's second half and
        # phase D's T-matmuls. ----
        twp = ctx.enter_context(tc.tile_pool(name="twp", bufs=6))
        cbp = ctx.enter_context(tc.tile_pool(name="cbp", bufs=3))
        bctx = ExitStack()
        ewp = bctx.enter_context(tc.tile_pool(name="ewp", bufs=8))
        ps_s = bctx.enter_context(tc.tile_pool(name="ps_s", bufs=3,
                                               space="PSUM"))
        ew_tiles = {}

        def b_dma(g):
            e_w = ewp.tile([L, GE, N_STATE], f16)
            nc.gpsimd.dma_start(out=e_w, in_=e_all[g])
            ew_tiles[g] = e_w

        def b_mms(g, k0, act_bias=False):
            e_w = ew_tiles[g]
            for i in range(GE // 4):            # 2 q (4 channels) per PSUM
                q0 = (g * GE) // 2 + 2 * i
                sp = ps_s.tile([128, 2, BATCH, NCH - 1], f32)
                for k in range(2):
                    for par in range(2):
                        dl = 2 * (q0 + k) + par
                        nc.tensor.matmul(
                            sp[64 * par:64 * (par + 1), k],
                            lhsT=e_w[:, 4 * i + 2 * k + par, :],
                            rhs=uy[:, :, 0:NCH - 1, dl],
                            start=True, stop=True,
                            tile_position=(0, 64 * par) if par else None)
                # while the DVE runs the half-scans, drain mostly via Act
                kk = 1 if act_bias else k0 + i
                cp(kk, x_src[:, :, q0:q0 + 2, 2:NCH + 1],
                   sp.transpose([0, 2, 1, 3]))
            return k0 + GE // 4

        def quarter_scans(k):
            # walrus: TensorScalarPtr is DVE-only. Quarter k covers q
            # 16k..16k+15 and only needs e-groups 2k and 2k+1 done, so each
            # quarter is emitted as early as its data allows and finishes
            # well before phase D's Cb matmuls reach its q range.
            qs = slice(16 * k, 16 * (k + 1))
            for b in range(BATCH):
                nc.vector.tensor_tensor_scan(
                    out=x_sc[:, b, qs].opt(), data0=pm_sb[:, qs].opt(),
                    data1=x_src[:, b, qs].opt(), initial=0.0,
                    op0=mybir.AluOpType.mult, op1=mybir.AluOpType.add)

        for g in range(8):                      # prefetch all E weights
            b_dma(g)
        for g in range(8):
            ncp = b_mms(g, ncp, act_bias=(g >= 2))
            if g % 2 == 1:
                quarter_scans(g // 2)
        # prefetch D weights for q<24 ahead of phase D
        tws, cbs = [], []
        for g in range(6):
            t_w = twp.tile([L, GT, L], f16)
            nc.gpsimd.dma_start(out=t_w, in_=t_all[g])
            tws.append(t_w)
        for g in range(3):
            cb_w = cbp.tile([128, GQ, L], f16)
            nc.gpsimd.dma_start(out=cb_w, in_=cb_all[g])
            cbs.append(cb_w)
        bctx.close()                            # release ewp + ps_s

        # ---- Phase D: y = T^T u (+) Cb^T h_start, overwrites uy ----
        with tc.tile_pool(name="ps_y", bufs=8, space="PSUM") as ps_y:
            for q in range(64):
                if q % (GT // 2) == 0:
                    if q < 24:
                        t_w = tws[q // 4]
                    else:
                        t_w = twp.tile([L, GT, L], f16)
                        nc.gpsimd.dma_start(out=t_w, in_=t_all[(2 * q) // GT])
                if q % GQ == 0:
                    if q < 24:
                        cb_w = cbs[q // 8]
                    else:
                        cb_w = cbp.tile([128, GQ, L], f16)
                        nc.gpsimd.dma_start(out=cb_w, in_=cb_all[q // GQ])
                yp = ps_y.tile([128, 2, BATCH, NCH], f32)
                for par in range(2):
                    # open+close one accumulation group per bank half —
                    # two simultaneously-open groups in one PSUM zero
                    # region are illegal.
                    dl = 2 * q + par
                    nc.tensor.matmul(yp[:, par], lhsT=t_w[:, dl % GT, :],
                                     rhs=uy[:, :, :, dl],
                                     start=True, stop=False)
                    nc.tensor.matmul(
                        yp[:, par],
                        lhsT=cb_w[64 * par:64 * (par + 1), q % GQ, :],
                        rhs=x_sc[64 * par:64 * (par + 1), :, q, 1:NCH + 1],
                        start=False, stop=True,
                        tile_position=(64 * par, 0) if par else None)
                # 3:5 DVE:Act split — the DVE still owes the h1 scans here
                cp(0 if q % 8 >= 5 else 1, uy[:, :, :, 2 * q:2 * q + 2],
                   yp.transpose([0, 2, 3, 1]))
                ncp += 1

        # ---- Phase E: transpose y + output projection -> out_t ----
        # Even G: PE transpose via PSUM; odd G: XBAR DMA transpose on the
        # otherwise-idle SP sequencer (SBUF->SBUF, no PSUM drain).
        with tc.tile_pool(name="ytp", bufs=4) as ytp, \
             tc.tile_pool(name="lop", bufs=3) as lop, \
             tc.tile_pool(name="ps_t", bufs=3, space="PSUM") as ps_t, \
             tc.tile_pool(name="ps_o", bufs=2, space="PSUM") as ps_o:
            for G in range(32):                 # 8 t-tiles per group
                yt = ytp.tile([128, 8, 128], f16)
                tp = ps_t.tile([128, 8, 128], f16)
                for k in range(8):
                    t = 8 * G + k
                    b, c = t // NCH, t % NCH
                    nc.tensor.transpose(tp[:, k, :], uy[:, b, c, :], ident)
                cp(0, yt, tp)
                ncp += 1
                # v-stacked projection: even G -> PSUM rows 0:64, odd G ->
                # rows 64:128 (weights loaded at PE columns 64:128), so one
                # [128, 1024] copy drains two G-groups of logits.
                if G % 4 == 0:
                    lo = lop.tile([128, 2, 8, L], bf16)   # 32 t
                if G % 2 == 0:
                    po = ps_o.tile([128, 2, 4 * L], f32)  # 2 banks
                sub = G % 2
                for h in range(2):
                    nc.tensor.matmul(po[64 * sub:64 * (sub + 1), h],
                                     lhsT=w2_sb,
                                     rhs=yt[:, 4 * h:4 * h + 4, :],
                                     start=True, stop=True,
                                     tile_position=(0, 64 * sub) if sub
                                     else None)
                if G % 2 == 1:
                    cp(1, lo[:, (G // 2) % 2], po)
                    ncp += 1
                if G % 4 == 3:
                    m2 = G // 4
                    # out col = v*32768 + 4096*m2 + 2048*mm + 1024*sub + f
                    full = out_t[:, m2 * 32 * L:(m2 + 1) * 32 * L] \
                        .rearrange("v (mm s f) -> v mm s f", mm=2, s=2)
                    if m2 < 7:
                        for sub in range(2):
                            eng = nc.sync if sub == 0 else nc.gpsimd
                            eng.dma_start(
                                out=full[:, :, sub, :],
                                in_=lo[64 * sub:64 * (sub + 1)].opt())
                    else:
                        # last group: 4 small DMAs across both queues so the
                        # first half ships while the last projections drain
                        for mm in range(2):
                            for sub in range(2):
                                eng = nc.sync if sub == 0 else nc.gpsimd
                                eng.dma_start(
                                    out=full[:, mm, sub, :],
                                    in_=lo[64 * sub:64 * (sub + 1), mm].opt())


def _build_nc():
    import concourse.tile as tile
    from concourse import bacc, mybir
    from concourse.masks import make_identity

    nc = bacc.Bacc(trn_type="TRN2", target_bir_lowering=False, debug=False)
    _emit_kernel(nc, tile, mybir, make_identity)
    nc.compile()
    return nc


_NC_CACHE = None


def _make_in_maps(x, emb, log_neg_A, B, C, D, log_dt, W_out, b_out):
    x = np.asarray(x).astype(np.int64)
    emb = np.asarray(emb, np.float32)
    log_neg_A = np.asarray(log_neg_A, np.float32)
    B_in = np.asarray(B, np.float32)
    C = np.asarray(C, np.float32)
    D_in = np.asarray(D, np.float32)
    log_dt = np.asarray(log_dt, np.float32)
    W_out = np.asarray(W_out, np.float32)

    T, E, Cb, P = _precompute_host(emb, log_neg_A, B_in, C, D_in, log_dt, W_out)

    # one-hot, token order col = (b*NCH + c)*L + j
    toks = x.reshape(BATCH, NCH, L).reshape(-1)
    onehotT = (np.arange(VOCAB)[:, None] == toks[None, :]).astype(np.float16)

    in_maps = []
    for core in range(NCORES):
        ds = slice(core * DPC, (core + 1) * DPC)
        # Pm layout [p=(par,n), q, c']: p = 64*par + n, d = 2*q + par
        Pc = P[ds].reshape(64, 2, N_STATE).transpose(1, 2, 0).reshape(128, 64)
        Pm = np.zeros((128, 64, NCH + 1), np.float16)
        Pm[:, :, 1:] = Pc[:, :, None].astype(np.float16)
        # t_all: [DPC,L,L] -> [DPC/GT, L, GT, L]
        Tc = np.ascontiguousarray(
            T[ds].reshape(DPC // GT, GT, L, L).transpose(0, 2, 1, 3))
        # e_all: [DPC,L,N] -> [DPC/GE, L, GE, N]
        Ec = np.ascontiguousarray(
            E[ds].reshape(DPC // GE, GE, L, N_STATE).transpose(0, 2, 1, 3))
        # cb_all: [DPC,N,L] -> pair-pack [64, 128=(par,n), L] -> groups of GQ
        Cbp = Cb[ds].reshape(64, 2 * N_STATE, L)   # [q, (par,n), L]
        Cbc = np.ascontiguousarray(
            Cbp.reshape(64 // GQ, GQ, 128, L).transpose(0, 2, 1, 3))
        in_maps.append({
            "onehot_t": onehotT,
            "emb_s": np.ascontiguousarray(emb[:, ds]).astype(np.float16),
            "t_all": Tc,
            "e_all": Ec,
            "cb_all": Cbc,
            "pm": Pm,
            "w2": np.ascontiguousarray(W_out[ds]).astype(np.float16),
        })
    return in_maps


def _postprocess(results, b_out):
    logitsT = np.zeros((VOCAB, BC * L), np.float64)
    for r in results:
        logitsT += r["out_t"].astype(np.float64)
    # col = (b*NCH + c)*L + j
    out = logitsT.T.reshape(BATCH, SEQ, VOCAB)
    return (out + np.asarray(b_out).astype(np.float64)).astype(np.float32)


def kernel(x, emb, log_neg_A, B, C, D, log_dt, W_out, b_out):
    global LAST_RESULTS, _NC_CACHE
    from concourse.bass_utils import run_bass_kernel_spmd

    in_maps = _make_in_maps(x, emb, log_neg_A, B, C, D, log_dt, W_out, b_out)

    if _NC_CACHE is None:
        _NC_CACHE = _build_nc()
    nc = _NC_CACHE

    trace = bool(int(os.environ.get("BASS_TRACE", "0") or "0"))
    LAST_RESULTS = run_bass_kernel_spmd(
        nc, in_maps, core_ids=list(range(NCORES)), trace=trace)

    return _postprocess(LAST_RESULTS.results, b_out)
